# revision 71
# baseline (speedup 1.0000x reference)
"""Trainium2 Bass kernel for a cross-attention transformer block.

Contract: kernel(**inputs) takes the FULL inputs (B=8 batch), shards
batch-wise across 8 NeuronCores (one batch element per core, SPMD, no
collectives), runs a Bass/Tile kernel, and returns the FULL output.

Per-core pipeline (everything stored feature-major, "X^T" [feat, tok],
so every linear is a single PE matmul pass with no transposes):
  Qp^T = (Wq/8)^T q^T   Kp^T = Wk^T k^T    (transposed-layout projections)
  Vp   = v^T-tiles as lhsT against Wv      (natural-layout projection,
                                            woven into the first head-pair
                                            of attention to fill the
                                            ACT-bound exp window)
  S^T  = Kp_h^T . Qp_h  per head (K=64)
  p    = exp(S^T + colNEG[k])              (one ACT op per (head, key-tile);
                                            no max subtraction: |s| <~ 5)
  out^T= [Vp | 1] @ p                      (M=65: row 64 = softmax denom)
  division: DVE reciprocal + GpSimd partition_broadcast + DVE multiply
  mha^T = Wo^T att^T, FFN with fused relu+bias, residual + LayerNorm
         in natural layout, DMA out.

The data path is bf16 (inputs/weights cast host-side): same PE matmul
rate as fp32r at >=256 wide, full rate at any width (no fp32r <256
penalty), half the DMA bytes, and f32 PSUM accumulation throughout.

Only query columns [0, q_pad) are computed on device. All-masked query
rows (>= per-batch valid len) produce one shared constant row
(uniform attention over every key); that row is computed exactly on the
host and patched into the result.
"""

import os
import sys

for _p in ("/opt/trn_rl_repo",):
    if _p not in sys.path:
        sys.path.insert(0, _p)

import numpy as np
import ml_dtypes

import concourse.bacc as bacc
import concourse.tile as tile
from concourse import mybir

F32 = mybir.dt.float32
F32R = mybir.dt.float32r
BF16 = mybir.dt.bfloat16
U16 = mybir.dt.uint16
AF = mybir.ActivationFunctionType
OP = mybir.AluOpType

D = 768
H = 12
HD = 64
DT = 6          # feature tiles of 128
L = 1024
NEG = -1000000.0
EPS = 1e-5
WARM_N = 4  # p-state warm-up chain length; keeps the PE ramp clock running
            # from t~0 so real matmuls are never charged at cold p-states
ONE_BF16 = 0x3F80


def _bchunks(w):
    """Column chunks that keep each matmul's PSUM output within one 2KB
    bank (<=512 f32)."""
    return [(0, 512), (512, w - 512)] if w > 512 else [(0, w)]


def _cdchunks(w):
    """Balanced <=512-wide 128-multiple chunks (for per-chunk PSUM tiles,
    where bank-crossing doesn't force the 512-boundary split)."""
    n = (w + 511) // 512
    out, off = [], 0
    for i in range(n):
        cw = ((w - off) // (n - i) + 127) // 128 * 128
        out.append((off, cw))
        off += cw
    return out


def _pad128(n):
    return int(min(L, max(256, ((int(n) + 127) // 128) * 128)))


def build_program(k_pad, q_pad, n_cores, has_g=True, has_b=True, has_d2b=True):
    kt_n = k_pad // 128
    qch = _bchunks(q_pad)
    cdch = _cdchunks(q_pad)
    kch = _bchunks(k_pad)
    qt_n = q_pad // 128

    nc = bacc.Bacc("TRN2", target_bir_lowering=False, debug=False,
                   num_devices=n_cores)

    def din(name, shape, dt=BF16):
        return nc.dram_tensor(name, shape, dt, kind="ExternalInput").ap()

    qT = din("qT", [D, q_pad])
    kT = din("kT", [D, k_pad])
    vT = din("vT", [D, k_pad])
    wq = din("wq", [D, D])
    wk = din("wk", [D, D])
    wv = din("wv", [D, D])
    wo = din("wo", [D, D])
    d1w = din("d1w", [D, D])
    d2w = din("d2w", [D, D])
    colneg = din("colneg", [128, kt_n], F32)
    pair01 = din("pair01", [65, 128])
    d1b = din("d1b", [128, DT], F32)
    d2b = din("d2b", [1, D], F32)
    lng = din("lng", [1, D], F32)
    lnb = din("lnb", [1, D], F32)
    out = nc.dram_tensor("out", [q_pad, D], F32, kind="ExternalOutput").ap()

    from contextlib import ExitStack
    with tile.TileContext(nc) as tc, ExitStack() as ctx:
        # ---------------- long-lived small tiles ----------------
        plong = ctx.enter_context(tc.tile_pool(name="plong", bufs=1))
        if WARM_N:
            # PE p-state warm-up: dependency-free back-to-back chain keeps
            # PE busy through the initial DMA wait so pe_busy_start is not
            # reset when the real matmuls arrive.
            sWarm = ExitStack()
            pwarm = sWarm.enter_context(tc.tile_pool(name="pwarm", bufs=1))
            pwarm_ps = sWarm.enter_context(
                tc.tile_pool(name="pwarm_ps", bufs=1, space="PSUM"))
            warm_a = pwarm.tile([1, 1], F32R, name="warm_a")
            warm_b = pwarm.tile([1, 512], F32R, name="warm_b")
            warm_ps = pwarm_ps.tile([1, 512], F32, name="warm_ps")
            nc.vector.memset(warm_a[:].bitcast(mybir.dt.uint32), 0x3F800000)
            nc.vector.memset(warm_b[:].bitcast(mybir.dt.uint32), 0x3F800000)
            for wi in range(WARM_N):
                nc.tensor.matmul(warm_ps[:, :], warm_a[:], warm_b[:],
                                 start=True, stop=True)
            sWarm.close()

        colneg_s = plong.tile([128, kt_n], F32, name="colneg_s")
        # head-pair reciprocal broadcast: stationary [65, 128] with row 0 ->
        # out partitions 0-63 and row 64 -> out partitions 64-127. The two
        # per-head reciprocals write rows 0 / 64 of a persistent [65, q_pad]
        # tile (engine writes must start at a 32-aligned partition; rows
        # 1-63 stay zero from a one-time memset so they contribute nothing).
        pairones = plong.tile([65, 128], BF16, name="pairones")
        nc.sync.dma_start(out=pairones[:], in_=pair01[:, :])
        rcP = [plong.tile([65, q_pad], BF16, name=f"rcP{i}") for i in (0, 1)]
        for i in (0, 1):
            nc.vector.memset(rcP[i][:].bitcast(U16), 0)
        gb = plong.tile([128, D], F32, name="gb") if has_g else None
        bb = plong.tile([128, D], F32, name="bb") if has_b else None
        d2bb = plong.tile([128, D], F32, name="d2bb") if has_d2b else None
        epst = plong.tile([128, 1], F32, name="epst")
        d1b_s = plong.tile([128, DT], F32, name="d1b_s")

        # attnorm^T lives from attention through the Wo projections
        sBC = ExitStack()
        pbc = sBC.enter_context(tc.tile_pool(name="pbc", bufs=1))
        attnorm = [pbc.tile([128, q_pad], BF16, name=f"attnorm{j}")
                   for j in range(DT)]

        # ---------------- phase A+B scope ----------------
        sAB = ExitStack()
        pproj = sAB.enter_context(tc.tile_pool(name="pproj", bufs=1))
        Qp = [pproj.tile([128, q_pad], BF16, name=f"Qp{j}") for j in range(DT)]
        Kp = [pproj.tile([128, k_pad], BF16, name=f"Kp{j}") for j in range(DT)]
        Vm65 = [pproj.tile([128, 65 * H], BF16, name=f"Vm65_{k}")
                for k in range(kt_n)]
        pinV = sAB.enter_context(tc.tile_pool(name="pinV", bufs=1))
        vTb = pinV.tile([128, DT * k_pad], BF16, name="vTb")
        wvb = pinV.tile([128, DT * D], BF16, name="wvb")

        # ---------------- phase A: Q/K projections ----------------
        sA = ExitStack()
        pin = sA.enter_context(tc.tile_pool(name="pin", bufs=1))
        pw = sA.enter_context(tc.tile_pool(name="pw", bufs=6))
        psA = sA.enter_context(tc.tile_pool(name="psA", bufs=1, space="PSUM"))

        # Big fused loads: one SBUF tile holds all six 128-row blocks of a
        # [768, W] DRAM tensor via a 3D access pattern — one DMA instead of
        # 6-18, avoiding per-DMA HWDGE overhead + sem propagation.
        def big_load(dst, dram_ap, w, splits):
            # dst[p, t*w + c] = dram[128*t + p, c]
            src = dram_ap.rearrange("(t p) w -> p t w", p=128)
            dstv = dst[:].rearrange("p (t w) -> p t w", w=w)
            for (o, ww) in splits:
                nc.sync.dma_start(out=dstv[:, :, o:o + ww],
                                  in_=src[:, :, o:o + ww])

        qTb = pin.tile([128, DT * q_pad], BF16, name="qTb")
        kTb = pin.tile([128, DT * k_pad], BF16, name="kTb")

        def xsl(big, t, w, c0, cw):
            return big[:, t * w + c0:t * w + c0 + cw]

        # All input DMAs issued upfront in priority order so the serial DMA
        # device streams them while the PE consumes earlier ones. qTb is
        # split into t-pair chunks so the first projection matmul starts
        # ~3us earlier than a monolithic load would allow.
        def tsplit_load(dst, dram_ap, w):
            src = dram_ap.rearrange("(t p) w -> p t w", p=128)
            dstv = dst[:].rearrange("p (t w) -> p t w", w=w)
            for t0 in range(0, DT, 2):
                nc.sync.dma_start(out=dstv[:, t0:t0 + 2, :],
                                  in_=src[:, t0:t0 + 2, :])

        whs = {}
        for (nm, wdram) in (("q", wq), ("k", wk)):
            for jh in range(3):
                whs[nm, jh] = pw.tile([128, DT * 256], BF16, tag="wh",
                                      name=f"wh_{nm}_{jh}")
        # finest granularity up front: the first projection matmul only
        # needs wq's (t=0, jh=0) block and qT's t=0 slice
        wh0v = whs["q", 0][:].rearrange("p (t w) -> p t w", w=256)
        wq0v = wq[:, 0:256].rearrange("(t p) w -> p t w", p=128)
        nc.sync.dma_start(out=wh0v[:, 0:1, :], in_=wq0v[:, 0:1, :])
        qTv = qTb[:].rearrange("p (t w) -> p t w", w=q_pad)
        qsv = qT.rearrange("(t p) w -> p t w", p=128)
        nc.sync.dma_start(out=qTv[:, 0:1, :], in_=qsv[:, 0:1, :])
        nc.sync.dma_start(out=wh0v[:, 1:DT, :], in_=wq0v[:, 1:DT, :])
        nc.sync.dma_start(out=qTv[:, 1:2, :], in_=qsv[:, 1:2, :])
        nc.sync.dma_start(out=qTv[:, 2:4, :], in_=qsv[:, 2:4, :])
        nc.sync.dma_start(out=qTv[:, 4:DT, :], in_=qsv[:, 4:DT, :])
        for jh in (1, 2):
            big_load(whs["q", jh], wq[:, 256 * jh:256 * jh + 256],
                     256, [(0, 256)])
        big_load(whs["k", 0], wk[:, 0:256], 256, [(0, 256)])
        tsplit_load(kTb, kT, k_pad)
        for jh in (1, 2):
            big_load(whs["k", jh], wk[:, 256 * jh:256 * jh + 256],
                     256, [(0, 256)])
        nc.sync.dma_start(out=colneg_s[:], in_=colneg[:, :])
        big_load(vTb, vT, k_pad, [(0, k_pad)])
        big_load(wvb, wv, D, [(0, D)])

        # Q/K projections in transposed layout, two dout tiles at a time.
        for (nm, xb, xw, outts, chs) in (
                ("q", qTb, q_pad, Qp, qch), ("k", kTb, k_pad, Kp, kch)):
            for jh in range(3):
                wh = whs[nm, jh]
                pss = {}
                for jj in range(2):
                    for (c0, cw) in chs:
                        pss[jj, c0] = psA.tile(
                            [128, cw], F32, tag="A", bufs=8,
                            name=f"psA_{nm}_{jh}_{jj}_{c0}",
                            padded_shape=[128, 512])
                for t in range(DT):
                    for jj in range(2):
                        for (c0, cw) in chs:
                            nc.tensor.matmul(
                                pss[jj, c0][:, :],
                                xsl(wh, t, 256, 128 * jj, 128),
                                xsl(xb, t, xw, c0, cw),
                                start=(t == 0), stop=(t == DT - 1))
                for jj in range(2):
                    j = 2 * jh + jj
                    for (c0, cw) in chs:
                        nc.scalar.copy(out=outts[j][:, c0:c0 + cw],
                                       in_=pss[jj, c0][:, :])

        sA.close()

        # ------- phase C/D weight prefetch (issued while A/B compute) -------
        # These DMA starts must be emitted before phase B: the SP sequencer
        # and the serial DMA device process work in program order, so
        # emitting them inside phase C would leave C's first matmuls waiting
        # on the Wo load.
        sCD = ExitStack()
        pcd = sCD.enter_context(tc.tile_pool(name="pcd", bufs=1, side="right"))
        mhaT = [pcd.tile([128, q_pad], BF16, name=f"mhaT{j}")
                for j in range(DT)]
        mhaN = [pcd.tile([128, D], F32, name=f"mhaN{q}") for q in range(qt_n)]
        if has_g:
            nc.sync.dma_start(out=gb[:], in_=lng.to_broadcast([128, D]))
        if has_b:
            nc.sync.dma_start(out=bb[:], in_=lnb.to_broadcast([128, D]))
        if has_d2b:
            nc.sync.dma_start(out=d2bb[:], in_=d2b.to_broadcast([128, D]))
        nc.vector.memset(epst[:], EPS)
        nc.sync.dma_start(out=d1b_s[:], in_=d1b[:, :])
        from concourse.masks import make_identity
        ident = pcd.tile([128, 128], BF16, name="ident")
        nc.vector.memset(ident[:].bitcast(U16), 0)
        make_identity(nc, ident[:], nomemset=True)
        pd_w = sCD.enter_context(tc.tile_pool(name="pd_w", bufs=1,
                                              side="right"))
        d2b_t = pd_w.tile([128, DT * D], BF16, name="d2b_t")
        d1b_t = pd_w.tile([128, DT * D], BF16, name="d1b_t")
        sW = ExitStack()
        pc_w = sW.enter_context(tc.tile_pool(name="pc_w", bufs=1,
                                             side="right"))
        wob = pc_w.tile([128, DT * D], BF16, name="wob")
        src_wo = wo.rearrange("(t p) w -> p t w", p=128)
        dst_wo = wob[:].rearrange("p (t w) -> p t w", w=D)
        nc.sync.dma_start(out=dst_wo[:, :, 0:384], in_=src_wo[:, :, 0:384])
        nc.sync.dma_start(out=dst_wo[:, :, 384:768], in_=src_wo[:, :, 384:768])
        for (wdram, dst) in ((d1w, d1b_t), (d2w, d2b_t)):
            s3 = wdram.rearrange("(t p) w -> p t w", p=128)
            d3 = dst[:].rearrange("p (t w) -> p t w", w=D)
            nc.sync.dma_start(out=d3[:, :, 0:384], in_=s3[:, :, 0:384])
            nc.sync.dma_start(out=d3[:, :, 384:768], in_=s3[:, :, 384:768])

        # ---------------- phase B: attention (V-proj woven in) -------------
        ppexp = sAB.enter_context(tc.tile_pool(name="ppexp", bufs=6))
        pden = sAB.enter_context(tc.tile_pool(name="pden", bufs=1))
        psB = sAB.enter_context(tc.tile_pool(name="psB", bufs=1, space="PSUM"))
        # Software-pipelined: AV matmuls run one key-tile behind the score
        # matmuls so the PE never sits inside the sc->exp->AV semaphore
        # round-trip, and each head-pair's division drain is emitted inside
        # the NEXT head-pair's loop so it never blocks the PE stream.
        def emit_recips(hp, aos, act=False):
            rc2 = rcP[hp % 2]
            for hx in (0, 1):
                with nc.allow_low_precision(
                        reason="fp22 recip is ample for softmax denom"):
                    nc.vector.reciprocal(out=rc2[64 * hx:64 * hx + 1, :],
                                         in_=aos[hx][64:65, :])

        def emit_drain_rest(hp, aos, rbp_tag="sc", rbs_act=False,
                            divide=False):
            # paired (reciprocal|denominator) broadcast via one PE matmul
            rc2 = rcP[hp % 2]
            rbp = psB.tile([128, q_pad], F32, tag=rbp_tag, bufs=2,
                           name=f"rbp{hp}", padded_shape=[128, 1024])
            for (q0, qw) in qch:
                nc.tensor.matmul(rbp[:, q0:q0 + qw], pairones[:, :],
                                 rc2[:, q0:q0 + qw], start=True, stop=True)
            rbs = pden.tile([128, q_pad], F32, tag="rbs", bufs=2,
                            name=f"rbs{hp}")
            if rbs_act:
                nc.scalar.copy(out=rbs[:], in_=rbp[:, :])
            else:
                nc.vector.tensor_copy(out=rbs[:], in_=rbp[:, :])
            for hx in (0, 1):
                po = 64 * hx
                nc.vector.tensor_tensor(
                    out=attnorm[hp][po:po + 64, :],
                    in0=aos[hx][0:64, :], in1=rbs[po:po + 64, :],
                    op=OP.mult)

        def emit_drain(hp, aos, **kw):
            emit_recips(hp, aos, act=kw.pop("recip_act", False))
            emit_drain_rest(hp, aos, **kw)

        def emit_vproj(kt):
            # V projection for one key tile, woven into B's ACT-bound slack
            psv = psB.tile([128, D], F32, tag="ao", bufs=2,
                           name=f"psV{kt}", padded_shape=[128, 1024])
            for t in range(DT):
                for (n0, nw) in ((0, 512), (512, 256)):
                    nc.tensor.matmul(
                        psv[:, n0:n0 + nw],
                        xsl(vTb, t, k_pad, 128 * kt, 128),
                        xsl(wvb, t, D, n0, nw),
                        start=(t == 0), stop=(t == DT - 1))
            vsrc = psv[:, :].rearrange("p (h e) -> p h e", e=64)
            vdst = Vm65[kt][:].rearrange(
                "p (h e) -> p h e", e=65)[:, :, 0:64]
            nc.vector.tensor_copy(out=vdst, in_=vsrc)
            nc.vector.memset(
                Vm65[kt][:].bitcast(U16)
                .rearrange("p (h e) -> p h e", e=65)[:, :, 64:65],
                ONE_BF16)

        def emit_av(hp, aos, hx, kt, p):
            h = 2 * hp + hx
            for (q0, qw) in qch:
                nc.tensor.matmul(
                    aos[hx][:, q0:q0 + qw],
                    Vm65[kt][:, 65 * h:65 * h + 65],
                    p[:, q0:q0 + qw],
                    start=(kt == 0), stop=(kt == kt_n - 1))

        # AV matmuls run TWO head-pairs behind the score/exp stream (their
        # p tiles persist), so the V projections and division drains slot
        # into the PE slack of the ACT-bound exp window. The V-projection
        # PSUMs reuse the "ao" accumulator tag: all V tiles are allocated
        # in hp 0-1, strictly before the first AV accumulator pair (hp2).
        vslots = {}
        for i in range(kt_n):
            # V(i) at slot (2i // kt_n, 2i % kt_n): spread over hp 0-1,
            # ahead of its deadline AV(hp=0, i) during hp=2's slots.
            vslots.setdefault(divmod(2 * i, kt_n), []).append(i)

        def alloc_aos(ahp):
            return {hx: psB.tile([65, q_pad], F32, tag="ao", bufs=2,
                                 name=f"ao{ahp}_{hx}",
                                 padded_shape=[65, 1024])
                    for hx in (0, 1)}

        def av_schedule(kt_n):
            # host-hp slot -> list of kts: front-loaded (2 then 3 per slot)
            # so the division drain can fire as early as possible
            sched = {}
            kt = 0
            slot = 1
            while kt < kt_n:
                take = min(2 if slot == 1 else 3, kt_n - kt)
                sched[slot] = list(range(kt, kt + take))
                kt += take
                slot += 1
            return sched

        AVS = av_schedule(kt_n)
        DRAIN_SLOT = min(max(AVS) + 1, kt_n - 1)
        ps_hist = {}   # hp -> {kt: [(hx, p), ...]}
        aos_by = {}    # accumulated-but-undrained head-pairs
        for hp in range(DT):
            ahp = hp - 2   # head-pair whose AVs + drain run during this hp
            ps_hist[hp] = {}
            for kt in range(kt_n):
                for vkt in vslots.get((hp, kt), []):
                    emit_vproj(vkt)
                ps_hist[hp][kt] = []
                for hx in (0, 1):
                    h = 2 * hp + hx
                    po = 64 * hx
                    sc = psB.tile([128, q_pad], F32, tag="sc", bufs=2,
                                  name=f"sc{hp}_{hx}_{kt}",
                                  padded_shape=[128, 1024])
                    for (q0, qw) in qch:
                        nc.tensor.matmul(
                            sc[:, q0:q0 + qw],
                            Kp[hp][po:po + 64, 128 * kt:128 * kt + 128],
                            Qp[hp][po:po + 64, q0:q0 + qw],
                            start=True, stop=True)
                    p = ppexp.tile([128, q_pad], BF16, tag="p", bufs=24,
                                   name=f"p{h}_{kt}")
                    nc.scalar.activation(out=p[:], in_=sc[:, :],
                                         func=AF.Exp,
                                         bias=colneg_s[:, kt:kt + 1],
                                         scale=1.0)
                    ps_hist[hp][kt].append((hx, p))
                if ahp >= 0:
                    if kt == 1:
                        aos_by[ahp] = alloc_aos(ahp)
                    for akt in AVS.get(kt, []):
                        for (hx, p) in ps_hist[ahp][akt]:
                            emit_av(ahp, aos_by[ahp], hx, akt, p)
                    if kt == DRAIN_SLOT:
                        tailish = ahp == DT - 3
                        emit_drain(ahp, aos_by.pop(ahp), rbs_act=tailish)
            if ahp >= 0:
                del ps_hist[ahp]
        # tail: the last two head-pairs accumulate back to back — the second
        # pair's accumulators live on the now-idle "sc" tag so its AVs don't
        # wait for the first pair's drain; recips overlap the other pair's
        # AV matmuls and the rbs copies ride the idle ACT engine. Allocation
        # order on the sc tag (rbp3, rbp4, aos5) is chosen so nothing waits
        # on a buffer whose release depends on later instructions.
        rbp4 = psB.tile([128, q_pad], F32, tag="sc", bufs=2,
                        name="rbp4t", padded_shape=[128, 1024])
        aos5 = {hx: psB.tile([65, q_pad], F32, tag="sc", bufs=2,
                             name=f"ao{DT - 1}_{hx}",
                             padded_shape=[65, 1024])
                for hx in (0, 1)}
        aos4 = alloc_aos(DT - 2)
        rc4 = rcP[(DT - 2) % 2]
        rc5 = rcP[(DT - 1) % 2]

        def one_recip(rc, aos, hx):
            with nc.allow_low_precision(
                    reason="fp22 recip is ample for softmax denom"):
                nc.vector.reciprocal(out=rc[64 * hx:64 * hx + 1, :],
                                     in_=aos[hx][64:65, :])

        # hx-major with eager reciprocals: each head's recip runs while the
        # other head's AVs stream, and mults4 are queued on DVE before the
        # last head's recip so only attnorm[5]'s chain trails the final AV.
        for kt in range(kt_n):
            emit_av(DT - 2, aos4, 0, kt, ps_hist[DT - 2][kt][0][1])
        one_recip(rc4, aos4, 0)
        for kt in range(kt_n):
            emit_av(DT - 2, aos4, 1, kt, ps_hist[DT - 2][kt][1][1])
        one_recip(rc4, aos4, 1)
        for kt in range(kt_n):
            emit_av(DT - 1, aos5, 0, kt, ps_hist[DT - 1][kt][0][1])
        one_recip(rc5, aos5, 0)
        for (q0, qw) in qch:
            nc.tensor.matmul(rbp4[:, q0:q0 + qw], pairones[:, :],
                             rc4[:, q0:q0 + qw], start=True, stop=True)
        rbs4 = pden.tile([128, q_pad], F32, tag="rbs", bufs=2, name="rbs4")
        nc.scalar.copy(out=rbs4[:], in_=rbp4[:, :])
        for hx in (0, 1):
            po = 64 * hx
            nc.vector.tensor_tensor(
                out=attnorm[DT - 2][po:po + 64, :],
                in0=aos4[hx][0:64, :], in1=rbs4[po:po + 64, :],
                op=OP.mult)
        for kt in range(kt_n):
            emit_av(DT - 1, aos5, 1, kt, ps_hist[DT - 1][kt][1][1])
        one_recip(rc5, aos5, 1)
        rbp5 = psB.tile([128, q_pad], F32, tag="ao", bufs=2,
                        name="rbp5t", padded_shape=[128, 1024])
        for (q0, qw) in qch:
            nc.tensor.matmul(rbp5[:, q0:q0 + qw], pairones[:, :],
                             rc5[:, q0:q0 + qw], start=True, stop=True)
        rbs5 = pden.tile([128, q_pad], F32, tag="rbs", bufs=2, name="rbs5")
        nc.scalar.copy(out=rbs5[:], in_=rbp5[:, :])
        for hx in (0, 1):
            po = 64 * hx
            nc.vector.tensor_tensor(
                out=attnorm[DT - 1][po:po + 64, :],
                in0=aos5[hx][0:64, :], in1=rbs5[po:po + 64, :],
                op=OP.mult)

        sAB.close()

        # ---------------- phase C: Wo (transposed layout) ----------------
        sC = ExitStack()
        psC = sC.enter_context(tc.tile_pool(name="psC", bufs=1, space="PSUM"))
        for (q0, qw) in cdch:
            for j in range(DT):
                ps = psC.tile([128, qw], F32, tag="C", bufs=3,
                              name=f"psT{j}_{q0}", padded_shape=[128, 512])
                for t in range(DT):
                    nc.tensor.matmul(ps[:, :],
                                     wob[:, t * D + 128 * j:
                                         t * D + 128 * j + 128],
                                     attnorm[t][:, q0:q0 + qw],
                                     start=(t == 0), stop=(t == DT - 1))
                # DVE: phase C is PE-bound and its DVE is idle
                nc.vector.tensor_copy(out=mhaT[j][:, q0:q0 + qw],
                                      in_=ps[:, :])
        sC.close()
        sW.close()
        sBC.close()

        # ---------------- phase D: FFN + residual + LayerNorm ---------------
        sD = ExitStack()
        pdx = sD.enter_context(tc.tile_pool(name="pdx", bufs=1, side="right"))
        psmall = sD.enter_context(
            tc.tile_pool(name="psmall", bufs=8, side="right"))
        psD = sD.enter_context(tc.tile_pool(name="psD", bufs=1, space="PSUM"))

        reluT = [pdx.tile([128, q_pad], BF16, name=f"reluT{j}")
                 for j in range(DT)]

        inv_d = 1.0 / D

        def emit_ffn_ln(qi, fast=False):
            ps = psD.tile([128, D], F32, tag="D", bufs=3, name=f"psff{qi}",
                          padded_shape=[128, 768])
            x = pdx.tile([128, D], F32, tag="x", bufs=2, name=f"x{qi}")
            scr = pdx.tile([128, D], F32, tag="scr", bufs=2, name=f"scr{qi}")
            xsum = psmall.tile([128, 1], F32, tag="s1", name=f"xsum{qi}")
            xsq = psmall.tile([128, 1], F32, tag="s2", name=f"xsq{qi}")
            if fast:
                # drain-critical block: consume each d2 column group as soon
                # as its accumulation stops, so only the last 256 columns'
                # stats chain is exposed after the final matmul
                xsum_b = psmall.tile([128, 1], F32, tag="s1b",
                                     name=f"xsumb{qi}")
                xsq_b = psmall.tile([128, 1], F32, tag="s2b",
                                    name=f"xsqb{qi}")
                for (n0, nw) in ((0, 512), (512, 256)):
                    for t in range(DT):
                        nc.tensor.matmul(ps[:, n0:n0 + nw],
                                         reluT[t][:, 128 * qi:128 * qi + 128],
                                         d2b_t[:, t * D + n0:t * D + n0 + nw],
                                         start=(t == 0), stop=(t == DT - 1))
                    acc = xsum if n0 == 0 else xsum_b
                    accq = xsq if n0 == 0 else xsq_b
                    nc.vector.scalar_tensor_tensor(
                        out=x[:, n0:n0 + nw], in0=ps[:, n0:n0 + nw],
                        scalar=0.0, in1=mhaN[qi][:, n0:n0 + nw],
                        op0=OP.bypass, op1=OP.add, accum_out=acc[:])
                    nc.scalar.activation(out=scr[:, n0:n0 + nw],
                                         in_=x[:, n0:n0 + nw],
                                         func=AF.Square, accum_out=accq[:])
                nc.vector.tensor_tensor(out=xsum[:], in0=xsum[:],
                                        in1=xsum_b[:], op=OP.add)
                nc.vector.tensor_tensor(out=xsq[:], in0=xsq[:],
                                        in1=xsq_b[:], op=OP.add)
            else:
                for (n0, nw) in ((0, 512), (512, 256)):
                    for t in range(DT):
                        nc.tensor.matmul(ps[:, n0:n0 + nw],
                                         reluT[t][:, 128 * qi:128 * qi + 128],
                                         d2b_t[:, t * D + n0:t * D + n0 + nw],
                                         start=(t == 0), stop=(t == DT - 1))
                # fused: x = ffn + mha, xsum = row-sum(x), one DVE pass
                nc.vector.scalar_tensor_tensor(
                    out=x[:], in0=ps[:, :], scalar=0.0, in1=mhaN[qi][:],
                    op0=OP.bypass, op1=OP.add, accum_out=xsum[:])
                nc.scalar.activation(out=scr[:], in_=x[:], func=AF.Square,
                                     accum_out=xsq[:])
            mu = psmall.tile([128, 1], F32, tag="s3", name=f"mu{qi}")
            nc.vector.tensor_scalar_mul(out=mu[:], in0=xsum[:], scalar1=inv_d)
            var = psmall.tile([128, 1], F32, tag="s4", name=f"var{qi}")
            # var = xsq/D - mu^2  ==  (xsq*1/D) - mu*mu
            mu2 = psmall.tile([128, 1], F32, tag="s5", name=f"mu2{qi}")
            nc.vector.tensor_tensor(out=mu2[:], in0=mu[:], in1=mu[:],
                                    op=OP.mult)
            nc.vector.scalar_tensor_tensor(out=var[:], in0=xsq[:],
                                           scalar=inv_d, in1=mu2[:],
                                           op0=OP.mult, op1=OP.subtract)
            std = psmall.tile([128, 1], F32, tag="s6", name=f"std{qi}")
            nc.scalar.activation(out=std[:], in_=var[:], func=AF.Sqrt,
                                 bias=epst[:], scale=1.0)
            rstd = psmall.tile([128, 1], F32, tag="s7", name=f"rstd{qi}")
            nc.vector.reciprocal(out=rstd[:], in_=std[:])
            nmb = psmall.tile([128, 1], F32, tag="s8", name=f"nmb{qi}")
            nc.vector.scalar_tensor_tensor(out=nmb[:], in0=mu[:], scalar=-1.0,
                                           in1=rstd[:], op0=OP.mult,
                                           op1=OP.mult)
            # xn = x*rstd + (-mu*rstd); alternate ACT (per-partition scale +
            # bias Identity) and DVE to balance the two engines; optional
            # *g (DVE) and +b
            cur = scr
            if fast:
                # halves: the first half's DMA (emitted below) overlaps the
                # second half's normalize
                for (n0, nw) in ((0, 384), (384, 384)):
                    nc.scalar.activation(out=cur[:, n0:n0 + nw],
                                         in_=x[:, n0:n0 + nw],
                                         func=AF.Identity,
                                         bias=nmb[:], scale=rstd[:])
                    if n0 == 0 and not has_g and not has_b:
                        nc.sync.dma_start(
                            out=out[128 * qi:128 * qi + 128, 0:384],
                            in_=cur[:, 0:384])
            elif qi % 2 == 0:
                nc.scalar.activation(out=cur[:], in_=x[:], func=AF.Identity,
                                     bias=nmb[:], scale=rstd[:])
            else:
                nc.vector.tensor_scalar(out=cur[:], in0=x[:],
                                        scalar1=rstd[:], scalar2=nmb[:],
                                        op0=OP.mult, op1=OP.add)
            if has_g:
                nc.vector.tensor_tensor(out=x[:], in0=cur[:], in1=gb[:],
                                        op=OP.mult)
                cur = x
            if has_b:
                xo = pdx.tile([128, D], F32, tag="xo", bufs=2, name=f"xo{qi}")
                if qi >= qt_n - 2:
                    nc.vector.tensor_tensor(out=xo[:], in0=cur[:], in1=bb[:],
                                            op=OP.add)
                else:
                    nc.gpsimd.tensor_tensor(out=xo[:], in0=cur[:], in1=bb[:],
                                            op=OP.add)
                cur = xo
            # split the writeback so the first half's DMA overlaps the rest
            # of the chain (matters for the last block's exposed drain)
            if not (fast and not has_g and not has_b):
                nc.sync.dma_start(out=out[128 * qi:128 * qi + 128, 0:384],
                                  in_=cur[:, 0:384])
            nc.sync.dma_start(out=out[128 * qi:128 * qi + 128, 384:768],
                              in_=cur[:, 384:768])

        for (q0, qw) in cdch:
            for j in range(DT):
                ps = psD.tile([128, qw], F32, tag="D", bufs=3,
                              name=f"psd1_{j}_{q0}", padded_shape=[128, 768])
                for t in range(DT):
                    nc.tensor.matmul(ps[:, :],
                                     d1b_t[:, t * D + 128 * j:
                                           t * D + 128 * j + 128],
                                     mhaT[t][:, q0:q0 + qw],
                                     start=(t == 0), stop=(t == DT - 1))
                nc.scalar.activation(out=reluT[j][:, q0:q0 + qw],
                                     in_=ps[:, :], func=AF.Relu,
                                     bias=d1b_s[:, j:j + 1], scale=1.0)
            for qi in range(q0 // 128, (q0 + qw) // 128):
                # mha natural = blockwise PE transpose of mhaT, emitted here
                # so the transposes fill phase-D pipeline gaps instead of
                # gating d1T at the C->D seam
                for j in range(DT):
                    tp = psD.tile([128, 128], BF16, tag="tr", bufs=2,
                                  name=f"tp{qi}_{j}")
                    nc.tensor.transpose(
                        tp[:, :], mhaT[j][:, 128 * qi:128 * qi + 128],
                        ident[:])
                    dst = mhaN[qi][:, 128 * j:128 * j + 128]
                    if has_d2b:
                        nc.vector.tensor_tensor(
                            out=dst, in0=tp[:, :],
                            in1=d2bb[:, 128 * j:128 * j + 128], op=OP.add)
                    elif j % 2 == 0:
                        # split the copies across ACT and DVE
                        nc.scalar.copy(out=dst, in_=tp[:, :])
                    else:
                        nc.vector.tensor_copy(out=dst, in_=tp[:, :])
                emit_ffn_ln(qi, fast=(qi == qt_n - 1))
        sD.close()
        sCD.close()

    nc.compile()
    return nc


_PROGRAM_CACHE = {}


def _get_program(k_pad, q_pad, n_cores, has_g, has_b, has_d2b):
    key = (k_pad, q_pad, n_cores, has_g, has_b, has_d2b)
    if key not in _PROGRAM_CACHE:
        _PROGRAM_CACHE[key] = build_program(k_pad, q_pad, n_cores,
                                            has_g, has_b, has_d2b)
    return _PROGRAM_CACHE[key]


def _bf16(x):
    return np.asarray(x, np.float32).astype(ml_dtypes.bfloat16)


def make_in_map(b, k_pad, q_pad, queries, keys, values, mask_1,
                Wq, Wk, Wv, Wo, d1_w, d1_b, d2_w, d2_b, ln_g, ln_b):
    kt_n = k_pad // 128
    f32 = np.float32
    vl1 = int(np.count_nonzero(mask_1[b]))
    col01 = (np.arange(L) < vl1)
    cn = np.where(col01, 0.0, NEG).astype(f32)[:k_pad]
    return {
        "qT": np.ascontiguousarray(
            _bf16(np.asarray(queries[b], f32) * 0.125).T[:, :q_pad]),
        "kT": np.ascontiguousarray(_bf16(keys[b]).T[:, :k_pad]),
        "vT": np.ascontiguousarray(_bf16(values[b]).T[:, :k_pad]),
        "wq": np.ascontiguousarray(_bf16(Wq)),
        "wk": np.ascontiguousarray(_bf16(Wk)),
        "wv": np.ascontiguousarray(_bf16(Wv)),
        "wo": np.ascontiguousarray(_bf16(Wo)),
        "d1w": np.ascontiguousarray(_bf16(d1_w)),
        "d2w": np.ascontiguousarray(_bf16(d2_w)),
        "colneg": np.ascontiguousarray(cn.reshape(kt_n, 128).T),
        "pair01": _pair01(),
        "d1b": np.ascontiguousarray(np.asarray(d1_b, f32).reshape(DT, 128).T),
        "d2b": np.ascontiguousarray(np.asarray(d2_b, f32)[None, :]),
        "lng": np.ascontiguousarray(np.asarray(ln_g, f32)[None, :]),
        "lnb": np.ascontiguousarray(np.asarray(ln_b, f32)[None, :]),
    }


def _pair01():
    p = np.zeros((65, 128), ml_dtypes.bfloat16)
    p[0, 0:64] = 1.0
    p[64, 64:128] = 1.0
    return p


def _masked_row(vb, Wv, Wo, d1_w, d1_b, d2_w, d2_b, ln_g, ln_b):
    """Exact output row for an all-masked query: uniform attention over
    every key (reference softmaxes equal NEG scores)."""
    f64 = np.float64
    att = np.asarray(vb, f64).mean(axis=0) @ np.asarray(Wv, f64)
    mha = att @ np.asarray(Wo, f64)
    h = np.maximum(mha @ np.asarray(d1_w, f64) + np.asarray(d1_b, f64), 0.0)
    ffn = h @ np.asarray(d2_w, f64) + np.asarray(d2_b, f64)
    x = ffn + mha
    mu = x.mean()
    var = np.square(x - mu).mean()
    ln = (x - mu) / np.sqrt(var + EPS) * np.asarray(ln_g, f64) \
        + np.asarray(ln_b, f64)
    return ln.astype(np.float32)


def kernel(queries, keys, values, mask_1, mask_2,
           Wq, Wk, Wv, Wo, d1_w, d1_b, d2_w, d2_b, ln_g, ln_b):
    from concourse.bass_utils import run_bass_kernel_spmd

    queries = np.asarray(queries)
    B = queries.shape[0]
    vl1 = np.count_nonzero(np.asarray(mask_1), axis=1)
    vl2 = np.count_nonzero(np.asarray(mask_2), axis=1)
    k_pad = _pad128(vl1.max())
    q_pad = _pad128(vl2.max())
    has_g = not np.all(np.asarray(ln_g) == 1.0)
    has_b = bool(np.any(np.asarray(ln_b)))
    has_d2b = bool(np.any(np.asarray(d2_b)))
    nc = _get_program(k_pad, q_pad, B, has_g, has_b, has_d2b)
    in_maps = [
        make_in_map(b, k_pad, q_pad, queries, keys, values, mask_1,
                    Wq, Wk, Wv, Wo, d1_w, d1_b, d2_w, d2_b, ln_g, ln_b)
        for b in range(B)
    ]
    res = run_bass_kernel_spmd(nc, in_maps, list(range(B)))
    full = np.empty((B, L, D), np.float32)
    for b in range(B):
        full[b, :q_pad, :] = res.results[b]["out"]
        # all-masked query rows share one constant row; patch it in exactly
        row = _masked_row(values[b], Wv, Wo, d1_w, d1_b, d2_w, d2_b,
                          ln_g, ln_b)
        full[b, int(vl2[b]):, :] = row[None, :]
    return full


# revision 72
# speedup vs baseline: 1.0042x; 1.0042x over previous
"""Trainium2 Bass kernel for a cross-attention transformer block.

Contract: kernel(**inputs) takes the FULL inputs (B=8 batch), shards
batch-wise across 8 NeuronCores (one batch element per core, SPMD, no
collectives), runs a Bass/Tile kernel, and returns the FULL output.

Per-core pipeline (everything stored feature-major, "X^T" [feat, tok],
so every linear is a single PE matmul pass with no transposes):
  Qp^T = (Wq/8)^T q^T   Kp^T = Wk^T k^T    (transposed-layout projections)
  Vp   = v^T-tiles as lhsT against Wv      (natural-layout projection,
                                            woven into the first head-pair
                                            of attention to fill the
                                            ACT-bound exp window)
  S^T  = Kp_h^T . Qp_h  per head (K=64)
  p    = exp(S^T + colNEG[k])              (one ACT op per (head, key-tile);
                                            no max subtraction: |s| <~ 5)
  out^T= [Vp | 1] @ p                      (M=65: row 64 = softmax denom)
  division: DVE reciprocal + GpSimd partition_broadcast + DVE multiply
  mha^T = Wo^T att^T, FFN with fused relu+bias, residual + LayerNorm
         in natural layout, DMA out.

The data path is bf16 (inputs/weights cast host-side): same PE matmul
rate as fp32r at >=256 wide, full rate at any width (no fp32r <256
penalty), half the DMA bytes, and f32 PSUM accumulation throughout.

Only query columns [0, q_pad) are computed on device. All-masked query
rows (>= per-batch valid len) produce one shared constant row
(uniform attention over every key); that row is computed exactly on the
host and patched into the result.
"""

import os
import sys

for _p in ("/opt/trn_rl_repo",):
    if _p not in sys.path:
        sys.path.insert(0, _p)

import numpy as np
import ml_dtypes

import concourse.bacc as bacc
import concourse.tile as tile
from concourse import mybir

F32 = mybir.dt.float32
F32R = mybir.dt.float32r
BF16 = mybir.dt.bfloat16
U16 = mybir.dt.uint16
AF = mybir.ActivationFunctionType
OP = mybir.AluOpType

D = 768
H = 12
HD = 64
DT = 6          # feature tiles of 128
L = 1024
NEG = -1000000.0
EPS = 1e-5
WARM_N = 4  # p-state warm-up chain length; keeps the PE ramp clock running
            # from t~0 so real matmuls are never charged at cold p-states
ONE_BF16 = 0x3F80


def _bchunks(w):
    """Column chunks that keep each matmul's PSUM output within one 2KB
    bank (<=512 f32)."""
    return [(0, 512), (512, w - 512)] if w > 512 else [(0, w)]


def _cdchunks(w):
    """Balanced <=512-wide 128-multiple chunks (for per-chunk PSUM tiles,
    where bank-crossing doesn't force the 512-boundary split)."""
    n = (w + 511) // 512
    out, off = [], 0
    for i in range(n):
        cw = ((w - off) // (n - i) + 127) // 128 * 128
        out.append((off, cw))
        off += cw
    return out


def _pad128(n):
    return int(min(L, max(256, ((int(n) + 127) // 128) * 128)))


def build_program(k_pad, q_pad, n_cores, has_g=True, has_b=True, has_d2b=True):
    kt_n = k_pad // 128
    qch = _bchunks(q_pad)
    cdch = _cdchunks(q_pad)
    kch = _bchunks(k_pad)
    qt_n = q_pad // 128

    nc = bacc.Bacc("TRN2", target_bir_lowering=False, debug=False,
                   num_devices=n_cores)

    def din(name, shape, dt=BF16):
        return nc.dram_tensor(name, shape, dt, kind="ExternalInput").ap()

    qT = din("qT", [D, q_pad])
    kT = din("kT", [D, k_pad])
    vT = din("vT", [D, k_pad])
    wq = din("wq", [D, D])
    wk = din("wk", [D, D])
    wv = din("wv", [D, D])
    wo = din("wo", [D, D])
    d1w = din("d1w", [D, D])
    d2w = din("d2w", [D, D])
    colneg = din("colneg", [128, kt_n], F32)
    pair01 = din("pair01", [65, 128])
    d1b = din("d1b", [128, DT], F32)
    d2b = din("d2b", [1, D], F32)
    lng = din("lng", [1, D], F32)
    lnb = din("lnb", [1, D], F32)
    out = nc.dram_tensor("out", [q_pad, D], F32, kind="ExternalOutput").ap()

    from contextlib import ExitStack
    with tile.TileContext(nc) as tc, ExitStack() as ctx:
        # ---------------- long-lived small tiles ----------------
        plong = ctx.enter_context(tc.tile_pool(name="plong", bufs=1))
        if WARM_N:
            # PE p-state warm-up: dependency-free back-to-back chain keeps
            # PE busy through the initial DMA wait so pe_busy_start is not
            # reset when the real matmuls arrive.
            sWarm = ExitStack()
            pwarm = sWarm.enter_context(tc.tile_pool(name="pwarm", bufs=1))
            pwarm_ps = sWarm.enter_context(
                tc.tile_pool(name="pwarm_ps", bufs=1, space="PSUM"))
            warm_a = pwarm.tile([1, 1], F32R, name="warm_a")
            warm_b = pwarm.tile([1, 512], F32R, name="warm_b")
            warm_ps = pwarm_ps.tile([1, 512], F32, name="warm_ps")
            nc.vector.memset(warm_a[:].bitcast(mybir.dt.uint32), 0x3F800000)
            nc.vector.memset(warm_b[:].bitcast(mybir.dt.uint32), 0x3F800000)
            for wi in range(WARM_N):
                nc.tensor.matmul(warm_ps[:, :], warm_a[:], warm_b[:],
                                 start=True, stop=True)
            sWarm.close()

        colneg_s = plong.tile([128, kt_n], F32, name="colneg_s")
        # head-pair reciprocal broadcast: stationary [65, 128] with row 0 ->
        # out partitions 0-63 and row 64 -> out partitions 64-127. The two
        # per-head reciprocals write rows 0 / 64 of a persistent [65, q_pad]
        # tile (engine writes must start at a 32-aligned partition; rows
        # 1-63 stay zero from a one-time memset so they contribute nothing).
        pairones = plong.tile([65, 128], BF16, name="pairones")
        nc.sync.dma_start(out=pairones[:], in_=pair01[:, :])
        rcP = [plong.tile([65, q_pad], BF16, name=f"rcP{i}") for i in (0, 1)]
        for i in (0, 1):
            nc.vector.memset(rcP[i][:].bitcast(U16), 0)
        gb = plong.tile([128, D], F32, name="gb") if has_g else None
        bb = plong.tile([128, D], F32, name="bb") if has_b else None
        d2bb = plong.tile([128, D], F32, name="d2bb") if has_d2b else None
        epst = plong.tile([128, 1], F32, name="epst")
        d1b_s = plong.tile([128, DT], F32, name="d1b_s")

        # attnorm^T lives from attention through the Wo projections
        sBC = ExitStack()
        pbc = sBC.enter_context(tc.tile_pool(name="pbc", bufs=1))
        attnorm = [pbc.tile([128, q_pad], BF16, name=f"attnorm{j}")
                   for j in range(DT)]

        # ---------------- phase A+B scope ----------------
        sAB = ExitStack()
        pproj = sAB.enter_context(tc.tile_pool(name="pproj", bufs=1))
        Qp = [pproj.tile([128, q_pad], BF16, name=f"Qp{j}") for j in range(DT)]
        Kp = [pproj.tile([128, k_pad], BF16, name=f"Kp{j}") for j in range(DT)]
        Vm65 = [pproj.tile([128, 65 * H], BF16, name=f"Vm65_{k}")
                for k in range(kt_n)]
        pinV = sAB.enter_context(tc.tile_pool(name="pinV", bufs=1))
        vTb = pinV.tile([128, DT * k_pad], BF16, name="vTb")
        wvb = pinV.tile([128, DT * D], BF16, name="wvb")

        # ---------------- phase A: Q/K projections ----------------
        sA = ExitStack()
        pin = sA.enter_context(tc.tile_pool(name="pin", bufs=1))
        pw = sA.enter_context(tc.tile_pool(name="pw", bufs=6))
        psA = sA.enter_context(tc.tile_pool(name="psA", bufs=1, space="PSUM"))

        # Big fused loads: one SBUF tile holds all six 128-row blocks of a
        # [768, W] DRAM tensor via a 3D access pattern — one DMA instead of
        # 6-18, avoiding per-DMA HWDGE overhead + sem propagation.
        def big_load(dst, dram_ap, w, splits):
            # dst[p, t*w + c] = dram[128*t + p, c]
            src = dram_ap.rearrange("(t p) w -> p t w", p=128)
            dstv = dst[:].rearrange("p (t w) -> p t w", w=w)
            for (o, ww) in splits:
                nc.sync.dma_start(out=dstv[:, :, o:o + ww],
                                  in_=src[:, :, o:o + ww])

        qTb = pin.tile([128, DT * q_pad], BF16, name="qTb")
        kTb = pin.tile([128, DT * k_pad], BF16, name="kTb")

        def xsl(big, t, w, c0, cw):
            return big[:, t * w + c0:t * w + c0 + cw]

        # All input DMAs issued upfront in priority order so the serial DMA
        # device streams them while the PE consumes earlier ones. qTb is
        # split into t-pair chunks so the first projection matmul starts
        # ~3us earlier than a monolithic load would allow.
        def tsplit_load(dst, dram_ap, w):
            src = dram_ap.rearrange("(t p) w -> p t w", p=128)
            dstv = dst[:].rearrange("p (t w) -> p t w", w=w)
            for t0 in range(0, DT, 2):
                nc.sync.dma_start(out=dstv[:, t0:t0 + 2, :],
                                  in_=src[:, t0:t0 + 2, :])

        whs = {}
        for (nm, wdram) in (("q", wq), ("k", wk)):
            for jh in range(3):
                whs[nm, jh] = pw.tile([128, DT * 256], BF16, tag="wh",
                                      name=f"wh_{nm}_{jh}")
        big_load(whs["q", 0], wq[:, 0:256], 256, [(0, 256)])
        tsplit_load(qTb, qT, q_pad)
        for jh in (1, 2):
            big_load(whs["q", jh], wq[:, 256 * jh:256 * jh + 256],
                     256, [(0, 256)])
        big_load(whs["k", 0], wk[:, 0:256], 256, [(0, 256)])
        tsplit_load(kTb, kT, k_pad)
        for jh in (1, 2):
            big_load(whs["k", jh], wk[:, 256 * jh:256 * jh + 256],
                     256, [(0, 256)])
        nc.sync.dma_start(out=colneg_s[:], in_=colneg[:, :])
        big_load(vTb, vT, k_pad, [(0, k_pad)])
        big_load(wvb, wv, D, [(0, D)])

        # Q/K projections in transposed layout, two dout tiles at a time.
        for (nm, xb, xw, outts, chs) in (
                ("q", qTb, q_pad, Qp, qch), ("k", kTb, k_pad, Kp, kch)):
            for jh in range(3):
                wh = whs[nm, jh]
                pss = {}
                for jj in range(2):
                    for (c0, cw) in chs:
                        pss[jj, c0] = psA.tile(
                            [128, cw], F32, tag="A", bufs=8,
                            name=f"psA_{nm}_{jh}_{jj}_{c0}",
                            padded_shape=[128, 512])
                for t in range(DT):
                    for jj in range(2):
                        for (c0, cw) in chs:
                            nc.tensor.matmul(
                                pss[jj, c0][:, :],
                                xsl(wh, t, 256, 128 * jj, 128),
                                xsl(xb, t, xw, c0, cw),
                                start=(t == 0), stop=(t == DT - 1))
                for jj in range(2):
                    j = 2 * jh + jj
                    for (c0, cw) in chs:
                        nc.scalar.copy(out=outts[j][:, c0:c0 + cw],
                                       in_=pss[jj, c0][:, :])

        sA.close()

        # ------- phase C/D weight prefetch (issued while A/B compute) -------
        # These DMA starts must be emitted before phase B: the SP sequencer
        # and the serial DMA device process work in program order, so
        # emitting them inside phase C would leave C's first matmuls waiting
        # on the Wo load.
        sCD = ExitStack()
        pcd = sCD.enter_context(tc.tile_pool(name="pcd", bufs=1, side="right"))
        mhaT = [pcd.tile([128, q_pad], BF16, name=f"mhaT{j}")
                for j in range(DT)]
        mhaN = [pcd.tile([128, D], F32, name=f"mhaN{q}") for q in range(qt_n)]
        if has_g:
            nc.sync.dma_start(out=gb[:], in_=lng.to_broadcast([128, D]))
        if has_b:
            nc.sync.dma_start(out=bb[:], in_=lnb.to_broadcast([128, D]))
        if has_d2b:
            nc.sync.dma_start(out=d2bb[:], in_=d2b.to_broadcast([128, D]))
        nc.vector.memset(epst[:], EPS)
        nc.sync.dma_start(out=d1b_s[:], in_=d1b[:, :])
        from concourse.masks import make_identity
        ident = pcd.tile([128, 128], BF16, name="ident")
        nc.vector.memset(ident[:].bitcast(U16), 0)
        make_identity(nc, ident[:], nomemset=True)
        pd_w = sCD.enter_context(tc.tile_pool(name="pd_w", bufs=1,
                                              side="right"))
        d2b_t = pd_w.tile([128, DT * D], BF16, name="d2b_t")
        d1b_t = pd_w.tile([128, DT * D], BF16, name="d1b_t")
        sW = ExitStack()
        pc_w = sW.enter_context(tc.tile_pool(name="pc_w", bufs=1,
                                             side="right"))
        wob = pc_w.tile([128, DT * D], BF16, name="wob")
        src_wo = wo.rearrange("(t p) w -> p t w", p=128)
        dst_wo = wob[:].rearrange("p (t w) -> p t w", w=D)
        nc.sync.dma_start(out=dst_wo[:, :, 0:384], in_=src_wo[:, :, 0:384])
        nc.sync.dma_start(out=dst_wo[:, :, 384:768], in_=src_wo[:, :, 384:768])
        for (wdram, dst) in ((d1w, d1b_t), (d2w, d2b_t)):
            s3 = wdram.rearrange("(t p) w -> p t w", p=128)
            d3 = dst[:].rearrange("p (t w) -> p t w", w=D)
            nc.sync.dma_start(out=d3[:, :, 0:384], in_=s3[:, :, 0:384])
            nc.sync.dma_start(out=d3[:, :, 384:768], in_=s3[:, :, 384:768])

        # ---------------- phase B: attention (V-proj woven in) -------------
        ppexp = sAB.enter_context(tc.tile_pool(name="ppexp", bufs=6))
        pden = sAB.enter_context(tc.tile_pool(name="pden", bufs=1))
        psB = sAB.enter_context(tc.tile_pool(name="psB", bufs=1, space="PSUM"))
        # Software-pipelined: AV matmuls run one key-tile behind the score
        # matmuls so the PE never sits inside the sc->exp->AV semaphore
        # round-trip, and each head-pair's division drain is emitted inside
        # the NEXT head-pair's loop so it never blocks the PE stream.
        def emit_recips(hp, aos, act=False):
            rc2 = rcP[hp % 2]
            for hx in (0, 1):
                with nc.allow_low_precision(
                        reason="fp22 recip is ample for softmax denom"):
                    nc.vector.reciprocal(out=rc2[64 * hx:64 * hx + 1, :],
                                         in_=aos[hx][64:65, :])

        def emit_drain_rest(hp, aos, rbp_tag="sc", rbs_act=False,
                            divide=False):
            # paired (reciprocal|denominator) broadcast via one PE matmul
            rc2 = rcP[hp % 2]
            rbp = psB.tile([128, q_pad], F32, tag=rbp_tag, bufs=2,
                           name=f"rbp{hp}", padded_shape=[128, 1024])
            for (q0, qw) in qch:
                nc.tensor.matmul(rbp[:, q0:q0 + qw], pairones[:, :],
                                 rc2[:, q0:q0 + qw], start=True, stop=True)
            rbs = pden.tile([128, q_pad], F32, tag="rbs", bufs=2,
                            name=f"rbs{hp}")
            if rbs_act:
                nc.scalar.copy(out=rbs[:], in_=rbp[:, :])
            else:
                nc.vector.tensor_copy(out=rbs[:], in_=rbp[:, :])
            for hx in (0, 1):
                po = 64 * hx
                nc.vector.tensor_tensor(
                    out=attnorm[hp][po:po + 64, :],
                    in0=aos[hx][0:64, :], in1=rbs[po:po + 64, :],
                    op=OP.mult)

        def emit_drain(hp, aos, **kw):
            emit_recips(hp, aos, act=kw.pop("recip_act", False))
            emit_drain_rest(hp, aos, **kw)

        def emit_vproj(kt):
            # V projection for one key tile, woven into B's ACT-bound slack
            psv = psB.tile([128, D], F32, tag="ao", bufs=2,
                           name=f"psV{kt}", padded_shape=[128, 1024])
            for t in range(DT):
                for (n0, nw) in ((0, 512), (512, 256)):
                    nc.tensor.matmul(
                        psv[:, n0:n0 + nw],
                        xsl(vTb, t, k_pad, 128 * kt, 128),
                        xsl(wvb, t, D, n0, nw),
                        start=(t == 0), stop=(t == DT - 1))
            vsrc = psv[:, :].rearrange("p (h e) -> p h e", e=64)
            vdst = Vm65[kt][:].rearrange(
                "p (h e) -> p h e", e=65)[:, :, 0:64]
            nc.vector.tensor_copy(out=vdst, in_=vsrc)
            nc.vector.memset(
                Vm65[kt][:].bitcast(U16)
                .rearrange("p (h e) -> p h e", e=65)[:, :, 64:65],
                ONE_BF16)

        def emit_av(hp, aos, hx, kt, p):
            h = 2 * hp + hx
            for (q0, qw) in qch:
                nc.tensor.matmul(
                    aos[hx][:, q0:q0 + qw],
                    Vm65[kt][:, 65 * h:65 * h + 65],
                    p[:, q0:q0 + qw],
                    start=(kt == 0), stop=(kt == kt_n - 1))

        # AV matmuls run TWO head-pairs behind the score/exp stream (their
        # p tiles persist), so the V projections and division drains slot
        # into the PE slack of the ACT-bound exp window. The V-projection
        # PSUMs reuse the "ao" accumulator tag: all V tiles are allocated
        # in hp 0-1, strictly before the first AV accumulator pair (hp2).
        vslots = {}
        for i in range(kt_n):
            # V(i) at slot (2i // kt_n, 2i % kt_n): spread over hp 0-1,
            # ahead of its deadline AV(hp=0, i) during hp=2's slots.
            vslots.setdefault(divmod(2 * i, kt_n), []).append(i)

        def alloc_aos(ahp):
            return {hx: psB.tile([65, q_pad], F32, tag="ao", bufs=2,
                                 name=f"ao{ahp}_{hx}",
                                 padded_shape=[65, 1024])
                    for hx in (0, 1)}

        def av_schedule(kt_n):
            # host-hp slot -> list of kts: front-loaded (2 then 3 per slot)
            # so the division drain can fire as early as possible
            sched = {}
            kt = 0
            slot = 1
            while kt < kt_n:
                take = min(2 if slot == 1 else 3, kt_n - kt)
                sched[slot] = list(range(kt, kt + take))
                kt += take
                slot += 1
            return sched

        AVS = av_schedule(kt_n)
        DRAIN_SLOT = min(max(AVS) + 1, kt_n - 1)
        ps_hist = {}   # hp -> {kt: [(hx, p), ...]}
        aos_by = {}    # accumulated-but-undrained head-pairs
        for hp in range(DT):
            ahp = hp - 2   # head-pair whose AVs + drain run during this hp
            ps_hist[hp] = {}
            for kt in range(kt_n):
                for vkt in vslots.get((hp, kt), []):
                    emit_vproj(vkt)
                ps_hist[hp][kt] = []
                for hx in (0, 1):
                    h = 2 * hp + hx
                    po = 64 * hx
                    sc = psB.tile([128, q_pad], F32, tag="sc", bufs=2,
                                  name=f"sc{hp}_{hx}_{kt}",
                                  padded_shape=[128, 1024])
                    for (q0, qw) in qch:
                        nc.tensor.matmul(
                            sc[:, q0:q0 + qw],
                            Kp[hp][po:po + 64, 128 * kt:128 * kt + 128],
                            Qp[hp][po:po + 64, q0:q0 + qw],
                            start=True, stop=True)
                    p = ppexp.tile([128, q_pad], BF16, tag="p", bufs=24,
                                   name=f"p{h}_{kt}")
                    nc.scalar.activation(out=p[:], in_=sc[:, :],
                                         func=AF.Exp,
                                         bias=colneg_s[:, kt:kt + 1],
                                         scale=1.0)
                    ps_hist[hp][kt].append((hx, p))
                if ahp >= 0:
                    if kt == 1:
                        aos_by[ahp] = alloc_aos(ahp)
                    for akt in AVS.get(kt, []):
                        for (hx, p) in ps_hist[ahp][akt]:
                            emit_av(ahp, aos_by[ahp], hx, akt, p)
                    if kt == DRAIN_SLOT:
                        tailish = ahp == DT - 3
                        emit_drain(ahp, aos_by.pop(ahp), rbs_act=tailish)
            if ahp >= 0:
                del ps_hist[ahp]
        # tail: the last two head-pairs accumulate back to back — the second
        # pair's accumulators live on the now-idle "sc" tag so its AVs don't
        # wait for the first pair's drain; recips overlap the other pair's
        # AV matmuls and the rbs copies ride the idle ACT engine. Allocation
        # order on the sc tag (rbp3, rbp4, aos5) is chosen so nothing waits
        # on a buffer whose release depends on later instructions.
        rbp4 = psB.tile([128, q_pad], F32, tag="sc", bufs=2,
                        name="rbp4t", padded_shape=[128, 1024])
        aos5 = {hx: psB.tile([65, q_pad], F32, tag="sc", bufs=2,
                             name=f"ao{DT - 1}_{hx}",
                             padded_shape=[65, 1024])
                for hx in (0, 1)}
        aos4 = alloc_aos(DT - 2)
        rc4 = rcP[(DT - 2) % 2]
        rc5 = rcP[(DT - 1) % 2]

        def one_recip(rc, aos, hx):
            with nc.allow_low_precision(
                    reason="fp22 recip is ample for softmax denom"):
                nc.vector.reciprocal(out=rc[64 * hx:64 * hx + 1, :],
                                     in_=aos[hx][64:65, :])

        # hx-major with eager reciprocals: each head's recip runs while the
        # other head's AVs stream, and mults4 are queued on DVE before the
        # last head's recip so only attnorm[5]'s chain trails the final AV.
        for kt in range(kt_n):
            emit_av(DT - 2, aos4, 0, kt, ps_hist[DT - 2][kt][0][1])
        one_recip(rc4, aos4, 0)
        for kt in range(kt_n):
            emit_av(DT - 2, aos4, 1, kt, ps_hist[DT - 2][kt][1][1])
        one_recip(rc4, aos4, 1)
        for kt in range(kt_n):
            emit_av(DT - 1, aos5, 0, kt, ps_hist[DT - 1][kt][0][1])
        one_recip(rc5, aos5, 0)
        for (q0, qw) in qch:
            nc.tensor.matmul(rbp4[:, q0:q0 + qw], pairones[:, :],
                             rc4[:, q0:q0 + qw], start=True, stop=True)
        rbs4 = pden.tile([128, q_pad], F32, tag="rbs", bufs=2, name="rbs4")
        nc.scalar.copy(out=rbs4[:], in_=rbp4[:, :])
        for hx in (0, 1):
            po = 64 * hx
            nc.vector.tensor_tensor(
                out=attnorm[DT - 2][po:po + 64, :],
                in0=aos4[hx][0:64, :], in1=rbs4[po:po + 64, :],
                op=OP.mult)
        for kt in range(kt_n):
            emit_av(DT - 1, aos5, 1, kt, ps_hist[DT - 1][kt][1][1])
        one_recip(rc5, aos5, 1)
        rbp5 = psB.tile([128, q_pad], F32, tag="ao", bufs=2,
                        name="rbp5t", padded_shape=[128, 1024])
        for (q0, qw) in qch:
            nc.tensor.matmul(rbp5[:, q0:q0 + qw], pairones[:, :],
                             rc5[:, q0:q0 + qw], start=True, stop=True)
        rbs5 = pden.tile([128, q_pad], F32, tag="rbs", bufs=2, name="rbs5")
        nc.scalar.copy(out=rbs5[:], in_=rbp5[:, :])
        for hx in (0, 1):
            po = 64 * hx
            nc.vector.tensor_tensor(
                out=attnorm[DT - 1][po:po + 64, :],
                in0=aos5[hx][0:64, :], in1=rbs5[po:po + 64, :],
                op=OP.mult)

        sAB.close()

        # ---------------- phase C: Wo (transposed layout) ----------------
        sC = ExitStack()
        psC = sC.enter_context(tc.tile_pool(name="psC", bufs=1, space="PSUM"))
        for (q0, qw) in cdch:
            for j in range(DT):
                ps = psC.tile([128, qw], F32, tag="C", bufs=3,
                              name=f"psT{j}_{q0}", padded_shape=[128, 512])
                for t in range(DT):
                    nc.tensor.matmul(ps[:, :],
                                     wob[:, t * D + 128 * j:
                                         t * D + 128 * j + 128],
                                     attnorm[t][:, q0:q0 + qw],
                                     start=(t == 0), stop=(t == DT - 1))
                # DVE: phase C is PE-bound and its DVE is idle
                nc.vector.tensor_copy(out=mhaT[j][:, q0:q0 + qw],
                                      in_=ps[:, :])
        sC.close()
        sW.close()
        sBC.close()

        # ---------------- phase D: FFN + residual + LayerNorm ---------------
        sD = ExitStack()
        pdx = sD.enter_context(tc.tile_pool(name="pdx", bufs=1, side="right"))
        psmall = sD.enter_context(
            tc.tile_pool(name="psmall", bufs=8, side="right"))
        psD = sD.enter_context(tc.tile_pool(name="psD", bufs=1, space="PSUM"))

        reluT = [pdx.tile([128, q_pad], BF16, name=f"reluT{j}")
                 for j in range(DT)]

        inv_d = 1.0 / D

        def emit_ffn_ln(qi, fast=False):
            ps = psD.tile([128, D], F32, tag="D", bufs=3, name=f"psff{qi}",
                          padded_shape=[128, 768])
            x = pdx.tile([128, D], F32, tag="x", bufs=2, name=f"x{qi}")
            scr = pdx.tile([128, D], F32, tag="scr", bufs=2, name=f"scr{qi}")
            xsum = psmall.tile([128, 1], F32, tag="s1", name=f"xsum{qi}")
            xsq = psmall.tile([128, 1], F32, tag="s2", name=f"xsq{qi}")
            if fast:
                # drain-critical block: consume each d2 column group as soon
                # as its accumulation stops, so only the last 256 columns'
                # stats chain is exposed after the final matmul
                xsum_b = psmall.tile([128, 1], F32, tag="s1b",
                                     name=f"xsumb{qi}")
                xsq_b = psmall.tile([128, 1], F32, tag="s2b",
                                    name=f"xsqb{qi}")
                for (n0, nw) in ((0, 512), (512, 256)):
                    for t in range(DT):
                        nc.tensor.matmul(ps[:, n0:n0 + nw],
                                         reluT[t][:, 128 * qi:128 * qi + 128],
                                         d2b_t[:, t * D + n0:t * D + n0 + nw],
                                         start=(t == 0), stop=(t == DT - 1))
                    acc = xsum if n0 == 0 else xsum_b
                    accq = xsq if n0 == 0 else xsq_b
                    nc.vector.scalar_tensor_tensor(
                        out=x[:, n0:n0 + nw], in0=ps[:, n0:n0 + nw],
                        scalar=0.0, in1=mhaN[qi][:, n0:n0 + nw],
                        op0=OP.bypass, op1=OP.add, accum_out=acc[:])
                    nc.scalar.activation(out=scr[:, n0:n0 + nw],
                                         in_=x[:, n0:n0 + nw],
                                         func=AF.Square, accum_out=accq[:])
                nc.vector.tensor_tensor(out=xsum[:], in0=xsum[:],
                                        in1=xsum_b[:], op=OP.add)
                nc.vector.tensor_tensor(out=xsq[:], in0=xsq[:],
                                        in1=xsq_b[:], op=OP.add)
            else:
                for (n0, nw) in ((0, 512), (512, 256)):
                    for t in range(DT):
                        nc.tensor.matmul(ps[:, n0:n0 + nw],
                                         reluT[t][:, 128 * qi:128 * qi + 128],
                                         d2b_t[:, t * D + n0:t * D + n0 + nw],
                                         start=(t == 0), stop=(t == DT - 1))
                # fused: x = ffn + mha, xsum = row-sum(x), one DVE pass
                nc.vector.scalar_tensor_tensor(
                    out=x[:], in0=ps[:, :], scalar=0.0, in1=mhaN[qi][:],
                    op0=OP.bypass, op1=OP.add, accum_out=xsum[:])
                nc.scalar.activation(out=scr[:], in_=x[:], func=AF.Square,
                                     accum_out=xsq[:])
            mu = psmall.tile([128, 1], F32, tag="s3", name=f"mu{qi}")
            nc.vector.tensor_scalar_mul(out=mu[:], in0=xsum[:], scalar1=inv_d)
            var = psmall.tile([128, 1], F32, tag="s4", name=f"var{qi}")
            # var = xsq/D - mu^2  ==  (xsq*1/D) - mu*mu
            mu2 = psmall.tile([128, 1], F32, tag="s5", name=f"mu2{qi}")
            nc.vector.tensor_tensor(out=mu2[:], in0=mu[:], in1=mu[:],
                                    op=OP.mult)
            nc.vector.scalar_tensor_tensor(out=var[:], in0=xsq[:],
                                           scalar=inv_d, in1=mu2[:],
                                           op0=OP.mult, op1=OP.subtract)
            std = psmall.tile([128, 1], F32, tag="s6", name=f"std{qi}")
            nc.scalar.activation(out=std[:], in_=var[:], func=AF.Sqrt,
                                 bias=epst[:], scale=1.0)
            rstd = psmall.tile([128, 1], F32, tag="s7", name=f"rstd{qi}")
            nc.vector.reciprocal(out=rstd[:], in_=std[:])
            nmb = psmall.tile([128, 1], F32, tag="s8", name=f"nmb{qi}")
            nc.vector.scalar_tensor_tensor(out=nmb[:], in0=mu[:], scalar=-1.0,
                                           in1=rstd[:], op0=OP.mult,
                                           op1=OP.mult)
            # xn = x*rstd + (-mu*rstd); alternate ACT (per-partition scale +
            # bias Identity) and DVE to balance the two engines; optional
            # *g (DVE) and +b
            cur = scr
            if fast:
                # halves: the first half's DMA (emitted below) overlaps the
                # second half's normalize
                for (n0, nw) in ((0, 384), (384, 384)):
                    nc.scalar.activation(out=cur[:, n0:n0 + nw],
                                         in_=x[:, n0:n0 + nw],
                                         func=AF.Identity,
                                         bias=nmb[:], scale=rstd[:])
                    if n0 == 0 and not has_g and not has_b:
                        nc.sync.dma_start(
                            out=out[128 * qi:128 * qi + 128, 0:384],
                            in_=cur[:, 0:384])
            elif qi % 2 == 0:
                nc.scalar.activation(out=cur[:], in_=x[:], func=AF.Identity,
                                     bias=nmb[:], scale=rstd[:])
            else:
                nc.vector.tensor_scalar(out=cur[:], in0=x[:],
                                        scalar1=rstd[:], scalar2=nmb[:],
                                        op0=OP.mult, op1=OP.add)
            if has_g:
                nc.vector.tensor_tensor(out=x[:], in0=cur[:], in1=gb[:],
                                        op=OP.mult)
                cur = x
            if has_b:
                xo = pdx.tile([128, D], F32, tag="xo", bufs=2, name=f"xo{qi}")
                if qi >= qt_n - 2:
                    nc.vector.tensor_tensor(out=xo[:], in0=cur[:], in1=bb[:],
                                            op=OP.add)
                else:
                    nc.gpsimd.tensor_tensor(out=xo[:], in0=cur[:], in1=bb[:],
                                            op=OP.add)
                cur = xo
            # split the writeback so the first half's DMA overlaps the rest
            # of the chain (matters for the last block's exposed drain)
            if not (fast and not has_g and not has_b):
                nc.sync.dma_start(out=out[128 * qi:128 * qi + 128, 0:384],
                                  in_=cur[:, 0:384])
            nc.sync.dma_start(out=out[128 * qi:128 * qi + 128, 384:768],
                              in_=cur[:, 384:768])

        for (q0, qw) in cdch:
            for j in range(DT):
                ps = psD.tile([128, qw], F32, tag="D", bufs=3,
                              name=f"psd1_{j}_{q0}", padded_shape=[128, 768])
                for t in range(DT):
                    nc.tensor.matmul(ps[:, :],
                                     d1b_t[:, t * D + 128 * j:
                                           t * D + 128 * j + 128],
                                     mhaT[t][:, q0:q0 + qw],
                                     start=(t == 0), stop=(t == DT - 1))
                nc.scalar.activation(out=reluT[j][:, q0:q0 + qw],
                                     in_=ps[:, :], func=AF.Relu,
                                     bias=d1b_s[:, j:j + 1], scale=1.0)
            for qi in range(q0 // 128, (q0 + qw) // 128):
                # mha natural = blockwise PE transpose of mhaT, emitted here
                # so the transposes fill phase-D pipeline gaps instead of
                # gating d1T at the C->D seam
                for j in range(DT):
                    tp = psD.tile([128, 128], BF16, tag="tr", bufs=2,
                                  name=f"tp{qi}_{j}")
                    nc.tensor.transpose(
                        tp[:, :], mhaT[j][:, 128 * qi:128 * qi + 128],
                        ident[:])
                    dst = mhaN[qi][:, 128 * j:128 * j + 128]
                    if has_d2b:
                        nc.vector.tensor_tensor(
                            out=dst, in0=tp[:, :],
                            in1=d2bb[:, 128 * j:128 * j + 128], op=OP.add)
                    elif j % 2 == 0:
                        # split the copies across ACT and DVE
                        nc.scalar.copy(out=dst, in_=tp[:, :])
                    else:
                        nc.vector.tensor_copy(out=dst, in_=tp[:, :])
                emit_ffn_ln(qi, fast=(qi == qt_n - 1))
        sD.close()
        sCD.close()

    nc.compile()
    return nc


_PROGRAM_CACHE = {}


def _get_program(k_pad, q_pad, n_cores, has_g, has_b, has_d2b):
    key = (k_pad, q_pad, n_cores, has_g, has_b, has_d2b)
    if key not in _PROGRAM_CACHE:
        _PROGRAM_CACHE[key] = build_program(k_pad, q_pad, n_cores,
                                            has_g, has_b, has_d2b)
    return _PROGRAM_CACHE[key]


def _bf16(x):
    return np.asarray(x, np.float32).astype(ml_dtypes.bfloat16)


def make_in_map(b, k_pad, q_pad, queries, keys, values, mask_1,
                Wq, Wk, Wv, Wo, d1_w, d1_b, d2_w, d2_b, ln_g, ln_b):
    kt_n = k_pad // 128
    f32 = np.float32
    vl1 = int(np.count_nonzero(mask_1[b]))
    col01 = (np.arange(L) < vl1)
    cn = np.where(col01, 0.0, NEG).astype(f32)[:k_pad]
    return {
        "qT": np.ascontiguousarray(
            _bf16(np.asarray(queries[b], f32) * 0.125).T[:, :q_pad]),
        "kT": np.ascontiguousarray(_bf16(keys[b]).T[:, :k_pad]),
        "vT": np.ascontiguousarray(_bf16(values[b]).T[:, :k_pad]),
        "wq": np.ascontiguousarray(_bf16(Wq)),
        "wk": np.ascontiguousarray(_bf16(Wk)),
        "wv": np.ascontiguousarray(_bf16(Wv)),
        "wo": np.ascontiguousarray(_bf16(Wo)),
        "d1w": np.ascontiguousarray(_bf16(d1_w)),
        "d2w": np.ascontiguousarray(_bf16(d2_w)),
        "colneg": np.ascontiguousarray(cn.reshape(kt_n, 128).T),
        "pair01": _pair01(),
        "d1b": np.ascontiguousarray(np.asarray(d1_b, f32).reshape(DT, 128).T),
        "d2b": np.ascontiguousarray(np.asarray(d2_b, f32)[None, :]),
        "lng": np.ascontiguousarray(np.asarray(ln_g, f32)[None, :]),
        "lnb": np.ascontiguousarray(np.asarray(ln_b, f32)[None, :]),
    }


def _pair01():
    p = np.zeros((65, 128), ml_dtypes.bfloat16)
    p[0, 0:64] = 1.0
    p[64, 64:128] = 1.0
    return p


def _masked_row(vb, Wv, Wo, d1_w, d1_b, d2_w, d2_b, ln_g, ln_b):
    """Exact output row for an all-masked query: uniform attention over
    every key (reference softmaxes equal NEG scores)."""
    f64 = np.float64
    att = np.asarray(vb, f64).mean(axis=0) @ np.asarray(Wv, f64)
    mha = att @ np.asarray(Wo, f64)
    h = np.maximum(mha @ np.asarray(d1_w, f64) + np.asarray(d1_b, f64), 0.0)
    ffn = h @ np.asarray(d2_w, f64) + np.asarray(d2_b, f64)
    x = ffn + mha
    mu = x.mean()
    var = np.square(x - mu).mean()
    ln = (x - mu) / np.sqrt(var + EPS) * np.asarray(ln_g, f64) \
        + np.asarray(ln_b, f64)
    return ln.astype(np.float32)


def kernel(queries, keys, values, mask_1, mask_2,
           Wq, Wk, Wv, Wo, d1_w, d1_b, d2_w, d2_b, ln_g, ln_b):
    from concourse.bass_utils import run_bass_kernel_spmd

    queries = np.asarray(queries)
    B = queries.shape[0]
    vl1 = np.count_nonzero(np.asarray(mask_1), axis=1)
    vl2 = np.count_nonzero(np.asarray(mask_2), axis=1)
    k_pad = _pad128(vl1.max())
    q_pad = _pad128(vl2.max())
    has_g = not np.all(np.asarray(ln_g) == 1.0)
    has_b = bool(np.any(np.asarray(ln_b)))
    has_d2b = bool(np.any(np.asarray(d2_b)))
    nc = _get_program(k_pad, q_pad, B, has_g, has_b, has_d2b)
    in_maps = [
        make_in_map(b, k_pad, q_pad, queries, keys, values, mask_1,
                    Wq, Wk, Wv, Wo, d1_w, d1_b, d2_w, d2_b, ln_g, ln_b)
        for b in range(B)
    ]
    res = run_bass_kernel_spmd(nc, in_maps, list(range(B)))
    full = np.empty((B, L, D), np.float32)
    for b in range(B):
        full[b, :q_pad, :] = res.results[b]["out"]
        # all-masked query rows share one constant row; patch it in exactly
        row = _masked_row(values[b], Wv, Wo, d1_w, d1_b, d2_w, d2_b,
                          ln_g, ln_b)
        full[b, int(vl2[b]):, :] = row[None, :]
    return full
